# revision 19
# baseline (speedup 1.0000x reference)
"""EventCameraSim Trainium2 kernel.

Math: per pixel p with It = log(initial_image+EPS), xl = log(x+EPS),
d = xl - It, the reference emits, for lane k in [1..48]:
    time_events[k] = (pol*k*C)/slope   if the k-th crossing is valid, else NaN
with slope = d/delta_t, pol = sign(d) gated on floor(|d/C|) > 0.

The valid mask reduces to a per-pixel prefix length n*:
  lane k valid  <=>  k <= n*,  where n* counts lanes with
  fl(pol*k*C + It) < xl (pol>0)  /  > xl (pol<0), gated on |d| >= C
(the gate floor(|fl(d/C)|)>0 is exactly equivalent to |d| >= 0.15f in fp32).
n* is computed EXACTLY with 3 boundary checks around ngu = round(|d|/C),
since lanes below/above that window are certain by a margin >> fp noise.
Valid lane values are k * (C*delta_t/|d|) up to a few ulp (tolerance-checked).

Device pipeline per core (90 of 720 rows, flat 345600 px = [128, 2700]):
  1. tiny XLA prelude: xl/It via jnp.log on the same neuron backend as the
     reference (bit-identical logs -> bit-exact NaN mask).
  2. Bass kernel: per-pixel stage (sign/gate/n*/scale, DVE+GPSIMD+ACT) then
     90 single-pass custom-DVE ops that expand each pixel to its 48-lane
     masked ramp [select(k<=n*, k*S, NaN)] and stream 66MB to HBM.

Sharding: H split across 8 cores; no communication.
"""

import sys
import numpy as np

for _p in ("/opt/trn_rl_repo",):
    if _p not in sys.path:
        sys.path.insert(0, _p)

H, W, CHN = 720, 1280, 3
K = 48
CTH = np.float32(0.15)
EPS = np.float32(1e-3)
T0 = np.float32(0.0)
NCORES = 8
ROWS_PER_CORE = H // NCORES          # 90
PIX_PER_CORE = ROWS_PER_CORE * W * CHN  # 345600
P = 128
COLS = PIX_PER_CORE // P             # 2700
CHUNKS = [180, 840, 840, 840]        # cols per chunk (divisible by PAGES; small first chunk hides DMA-in)
NCHUNK = len(CHUNKS)
PAGES = 60                           # pixels per partition-row per output tile (2 image rows)

_cache = {}


def _register_event_op():
    """Register the fused masked-ramp custom DVE op at runtime."""
    from concourse import dve_ops as _dv
    from concourse.dve_spec import (
        Spec, Src0, Src1, C0, C1, One, lower, Idx, PageIdx, select,
    )
    from concourse.dve_uop import DveOpSpec

    for op in _dv.OPS:
        if op.name == "EVENT_RAMP_MASK":
            return op
    k1 = Idx + PageIdx(One, C1)          # C1 = -K at call site -> k = 1..K per page
    body = select(k1 <= Src1, k1 * Src0, C0)

    def ref(in0, in1, s0, s1, imm2):
        x = np.asarray(in0)
        n3 = np.asarray(in1).reshape(x.shape[0], -1, K)
        x3 = x.reshape(x.shape[0], -1, K)
        k = np.arange(1, K + 1, dtype=np.float32)[None, None, :]
        return np.where(k <= n3, k * x3, np.float32(s0)).reshape(x.shape)

    spec = Spec(body=body, reference=ref)
    return _register_op("EVENT_RAMP_MASK", spec, subdim=True)


def _register_op(name, spec, subdim):
    from concourse import dve_ops as _dv
    from concourse.dve_spec import lower, _has_src1
    from concourse.dve_uop import DveOpSpec

    row = _dv._CUSTOM_DVE_ROW_BASE + len(_dv.OPS)
    assert row < 0x20
    _dv._SUB_OPCODE_FOR_NAME[name] = row
    shas = {}
    for ver in ("v3", "v4"):
        s = DveOpSpec(name=name, opcode=row, uops=lower(spec, ver=ver),
                      rd1_en=_has_src1(spec))
        shas[ver] = s.sha(ver)
    op = _dv.DveOp(name, spec, subdim=subdim, uops_sha=shas)
    _dv.OPS.append(op)
    _dv.CUSTOM_DVE_SPECS[name] = spec
    return op


def _register_w3_op():
    """w = ((in0 + s0) * s1) + in1 — the boundary-check lhs, same roundings
    as the reference's karange*CTH + It chain."""
    from concourse import dve_ops as _dv
    from concourse.dve_spec import Spec, Src0, Src1, C0, C1

    for op in _dv.OPS:
        if op.name == "EVENT_W3":
            return op
    spec = Spec(
        body=(Src0 + C0) * C1 + Src1,
        reference=lambda in0, in1, s0, s1, imm2: (
            ((in0 + np.float32(s0)) * np.float32(s1)) + in1).astype(np.float32),
    )
    return _register_op("EVENT_W3", spec, subdim=False)


def _build_bass():
    """Build + finalize the per-core SPMD Bass module (cached)."""
    if "nc" in _cache:
        return _cache["nc"]
    import concourse.bacc as bacc
    import concourse.mybir as mybir
    from concourse.tile import TileContext

    OP = _register_event_op()
    f32 = mybir.dt.float32
    Alu = mybir.AluOpType
    Act = mybir.ActivationFunctionType

    nc = bacc.Bacc()
    xl_in = nc.dram_tensor("xl", [P, COLS], f32, kind="ExternalInput")
    it_in = nc.dram_tensor("itl", [P, COLS], f32, kind="ExternalInput")
    cdt_in = nc.dram_tensor("cdt", [P, 1], f32, kind="ExternalInput")
    out_d = nc.dram_tensor("out", [P, COLS * K], f32, kind="ExternalOutput")

    with TileContext(nc) as tc:
        with tc.tile_pool(name="cst", bufs=1) as cst, \
             tc.tile_pool(name="pp", bufs=2) as pp, \
             tc.tile_pool(name="mp", bufs=6) as mp, \
             tc.tile_pool(name="pf", bufs=1) as pf:
            nan_t = cst.tile([P, 1], f32)
            nc.vector.memset(nan_t[:], float("nan"))
            cdt_t = cst.tile([P, 1], f32)
            nc.sync.dma_start(cdt_t[:], cdt_in[:])
            xls, its = [], []
            _off = 0
            for c, CHCOLS in enumerate(CHUNKS):
                sl = (slice(None), slice(_off, _off + CHCOLS))
                xl_c = pf.tile([P, CHCOLS], f32, tag=f"xl{c}")
                it_c = pf.tile([P, CHCOLS], f32, tag=f"it{c}")
                nc.sync.dma_start(xl_c[:], xl_in[sl])
                nc.sync.dma_start(it_c[:], it_in[sl])
                xls.append(xl_c); its.append(it_c)
                _off += CHCOLS
            cs = 0
            for c, CHCOLS in enumerate(CHUNKS):
                TILES_PER_CHUNK = CHCOLS // PAGES
                xl, it = xls[c], its[c]

                d = pp.tile([P, CHCOLS], f32, tag="d")
                nc.vector.tensor_tensor(d[:], xl[:], it[:], Alu.subtract)
                sg = pp.tile([P, CHCOLS], f32, tag="sg")
                nc.scalar.sign(sg[:], d[:])
                ad = pp.tile([P, CHCOLS], f32, tag="ad")
                nc.scalar.activation(ad[:], d[:], Act.Abs)         # |d|, exact
                A = pp.tile([P, CHCOLS], f32, tag="A")
                nc.vector.tensor_tensor(A[:], xl[:], sg[:], Alu.mult)
                B = pp.tile([P, CHCOLS], f32, tag="B")
                nc.vector.tensor_tensor(B[:], it[:], sg[:], Alu.mult)

                # exact gate as a clamp bound: pol != 0 <=> |d| >= 0.15f.
                # Sign(|d| - C) is sign-exact; {-1,0,1} -> {0,24,48}. The 0.15f
                # midpoint (Sign=0 -> 24) is safe: |d|=C implies n* <= 1.
                g48 = pp.tile([P, CHCOLS], f32, tag="g48")
                nc.scalar.activation(g48[:], ad[:], Act.Copy, bias=-float(CTH))
                nc.scalar.sign(g48[:], g48[:])
                nc.scalar.activation(g48[:], g48[:], Act.Copy, bias=24.0, scale=24.0)
                # window center: round(|d|/C) via the 2^23 trick (+-1 tolerant)
                nr = pp.tile([P, CHCOLS], f32, tag="nr")
                nc.scalar.activation(nr[:], ad[:], Act.Copy, bias=8388608.0,
                                     scale=float(np.float32(1.0) / CTH))
                nc.scalar.activation(nr[:], nr[:], Act.Copy, bias=-8388608.0)

                # three exact boundary checks at k = nr-1, nr, nr+1
                cbs = []
                for ji, jv in enumerate((-1.0, 0.0, 1.0)):
                    kc = pp.tile([P, CHCOLS], f32, tag=f"kc{ji}")
                    nc.vector.tensor_scalar(kc[:], nr[:], jv, float(CTH),
                                            Alu.add, Alu.mult)
                    nc.vector.tensor_tensor(kc[:], kc[:], B[:], Alu.add)
                    nc.vector.tensor_tensor(kc[:], kc[:], A[:], Alu.is_lt)
                    cbs.append(kc)
                ns = pp.tile([P, CHCOLS], f32, tag="ns")
                nc.vector.tensor_tensor(ns[:], cbs[0][:], cbs[1][:], Alu.add)
                npre = pp.tile([P, CHCOLS], f32, tag="npre")
                nc.vector.scalar_tensor_tensor(npre[:], nr[:], -2.0, ns[:],
                                               Alu.add, Alu.add)
                nc.vector.tensor_tensor(npre[:], npre[:], cbs[2][:], Alu.add)
                nstc = pp.tile([P, CHCOLS], f32, tag="nstc")
                nc.vector.tensor_tensor(nstc[:], npre[:], g48[:], Alu.min)

                # S = C*delta_t / |d|  (~2 ulp; garbage when masked-out is fine)
                scr = pp.tile([P, CHCOLS], f32, tag="scr")
                rec = pp.tile([P, CHCOLS], f32, tag="rec")
                nc.vector.reciprocal_approx_accurate(rec[:], ad[:], scr[:])
                S = pp.tile([P, CHCOLS], f32, tag="S")
                nc.scalar.mul(S[:], rec[:], cdt_t[:, 0:1])

                # main pass: one fused DVE op per image row
                for t in range(TILES_PER_CHUNK):
                    o_t = mp.tile([P, PAGES * K], f32, tag="o")
                    S_b = S[:, t * PAGES:(t + 1) * PAGES].broadcast_to([P, PAGES, K])
                    n_b = nstc[:, t * PAGES:(t + 1) * PAGES].broadcast_to([P, PAGES, K])
                    nc.vector._custom_dve(
                        OP, out=o_t[:].rearrange("p (s n) -> p s n", n=K),
                        in0=S_b, in1=n_b, s0=nan_t[:, 0:1], s1=float(-K))
                    o0 = (cs + t * PAGES) * K
                    nc.sync.dma_start(out_d[:, o0:o0 + PAGES * K], o_t[:])
                cs += CHCOLS

    nc.finalize()
    _cache["nc"] = nc
    return nc


def _log_prelude(x, initial_image):
    """xl/It via the same XLA log the reference uses (bit-identical)."""
    import jax, jax.numpy as jnp
    f = _cache.get("logf")
    if f is None:
        f = jax.jit(lambda a: jnp.log(a + EPS))
        _cache["logf"] = f
    It = np.asarray(f(np.asarray(initial_image, dtype=np.float32)))
    xl = np.asarray(f(np.asarray(x, dtype=np.float32)))
    return xl, It


def kernel(x, initial_image, time):
    from concourse.bass_utils import run_bass_kernel_spmd

    nc = _build_bass()
    xl, It = _log_prelude(x, initial_image)
    dt = np.float32(time) - T0
    cdt = np.full((P, 1), CTH * dt, dtype=np.float32)

    xl_f = xl.reshape(-1)
    it_f = It.reshape(-1)
    in_maps = []
    for c in range(NCORES):
        s = c * PIX_PER_CORE
        in_maps.append({
            "xl": xl_f[s:s + PIX_PER_CORE].reshape(P, COLS),
            "itl": it_f[s:s + PIX_PER_CORE].reshape(P, COLS),
            "cdt": cdt,
        })
    res = run_bass_kernel_spmd(nc, in_maps, core_ids=list(range(NCORES)))
    out = np.empty((H * W * CHN, K), dtype=np.float32)
    for c in range(NCORES):
        s = c * PIX_PER_CORE
        out[s:s + PIX_PER_CORE] = res.results[c]["out"].reshape(PIX_PER_CORE, K)
    return out.reshape(H, W, CHN, K)


# revision 20
# speedup vs baseline: 1.0050x; 1.0050x over previous
"""EventCameraSim Trainium2 kernel.

Math: per pixel p with It = log(initial_image+EPS), xl = log(x+EPS),
d = xl - It, the reference emits, for lane k in [1..48]:
    time_events[k] = (pol*k*C)/slope   if the k-th crossing is valid, else NaN
with slope = d/delta_t, pol = sign(d) gated on floor(|d/C|) > 0.

The valid mask reduces to a per-pixel prefix length n*:
  lane k valid  <=>  k <= n*,  where n* counts lanes with
  fl(pol*k*C + It) < xl (pol>0)  /  > xl (pol<0), gated on |d| >= C
(the gate floor(|fl(d/C)|)>0 is exactly equivalent to |d| >= 0.15f in fp32).
n* is computed EXACTLY with 3 boundary checks around ngu = round(|d|/C),
since lanes below/above that window are certain by a margin >> fp noise.
Valid lane values are k * (C*delta_t/|d|) up to a few ulp (tolerance-checked).

Device pipeline per core (90 of 720 rows, flat 345600 px = [128, 2700]):
  1. tiny XLA prelude: xl/It via jnp.log on the same neuron backend as the
     reference (bit-identical logs -> bit-exact NaN mask).
  2. Bass kernel: per-pixel stage (sign/gate/n*/scale, DVE+GPSIMD+ACT) then
     90 single-pass custom-DVE ops that expand each pixel to its 48-lane
     masked ramp [select(k<=n*, k*S, NaN)] and stream 66MB to HBM.

Sharding: H split across 8 cores; no communication.
"""

import sys
import numpy as np

for _p in ("/opt/trn_rl_repo",):
    if _p not in sys.path:
        sys.path.insert(0, _p)

H, W, CHN = 720, 1280, 3
K = 48
CTH = np.float32(0.15)
EPS = np.float32(1e-3)
T0 = np.float32(0.0)
NCORES = 8
ROWS_PER_CORE = H // NCORES          # 90
PIX_PER_CORE = ROWS_PER_CORE * W * CHN  # 345600
P = 128
COLS = PIX_PER_CORE // P             # 2700
CHUNKS = [180, 840, 840, 840]        # cols per chunk (divisible by PAGES; small first chunk hides DMA-in)
NCHUNK = len(CHUNKS)
PAGES = 60                           # pixels per partition-row per output tile (2 image rows)

_cache = {}


def _register_event_op():
    """Register the fused masked-ramp custom DVE op at runtime."""
    from concourse import dve_ops as _dv
    from concourse.dve_spec import (
        Spec, Src0, Src1, C0, C1, One, lower, Idx, PageIdx, select,
    )
    from concourse.dve_uop import DveOpSpec

    for op in _dv.OPS:
        if op.name == "EVENT_RAMP_MASK":
            return op
    k1 = Idx + PageIdx(One, C1)          # C1 = -K at call site -> k = 1..K per page
    body = select(k1 <= Src1, k1 * Src0, C0)

    def ref(in0, in1, s0, s1, imm2):
        x = np.asarray(in0)
        n3 = np.asarray(in1).reshape(x.shape[0], -1, K)
        x3 = x.reshape(x.shape[0], -1, K)
        k = np.arange(1, K + 1, dtype=np.float32)[None, None, :]
        return np.where(k <= n3, k * x3, np.float32(s0)).reshape(x.shape)

    spec = Spec(body=body, reference=ref)
    return _register_op("EVENT_RAMP_MASK", spec, subdim=True)


def _register_op(name, spec, subdim):
    from concourse import dve_ops as _dv
    from concourse.dve_spec import lower, _has_src1
    from concourse.dve_uop import DveOpSpec

    row = _dv._CUSTOM_DVE_ROW_BASE + len(_dv.OPS)
    assert row < 0x20
    _dv._SUB_OPCODE_FOR_NAME[name] = row
    shas = {}
    for ver in ("v3", "v4"):
        s = DveOpSpec(name=name, opcode=row, uops=lower(spec, ver=ver),
                      rd1_en=_has_src1(spec))
        shas[ver] = s.sha(ver)
    op = _dv.DveOp(name, spec, subdim=subdim, uops_sha=shas)
    _dv.OPS.append(op)
    _dv.CUSTOM_DVE_SPECS[name] = spec
    return op


def _register_w3_op():
    """w = ((in0 + s0) * s1) + in1 — the boundary-check lhs, same roundings
    as the reference's karange*CTH + It chain."""
    from concourse import dve_ops as _dv
    from concourse.dve_spec import Spec, Src0, Src1, C0, C1

    for op in _dv.OPS:
        if op.name == "EVENT_W3":
            return op
    spec = Spec(
        body=(Src0 + C0) * C1 + Src1,
        reference=lambda in0, in1, s0, s1, imm2: (
            ((in0 + np.float32(s0)) * np.float32(s1)) + in1).astype(np.float32),
    )
    return _register_op("EVENT_W3", spec, subdim=False)


def _build_bass():
    """Build + finalize the per-core SPMD Bass module (cached)."""
    if "nc" in _cache:
        return _cache["nc"]
    import concourse.bacc as bacc
    import concourse.mybir as mybir
    from concourse.tile import TileContext

    OP = _register_event_op()
    f32 = mybir.dt.float32
    Alu = mybir.AluOpType
    Act = mybir.ActivationFunctionType

    nc = bacc.Bacc()
    xl_in = nc.dram_tensor("xl", [P, COLS], f32, kind="ExternalInput")
    it_in = nc.dram_tensor("itl", [P, COLS], f32, kind="ExternalInput")
    cdt_in = nc.dram_tensor("cdt", [P, 1], f32, kind="ExternalInput")
    out_d = nc.dram_tensor("out", [P, COLS * K], f32, kind="ExternalOutput")

    with TileContext(nc) as tc:
        with tc.tile_pool(name="cst", bufs=1) as cst, \
             tc.tile_pool(name="pp", bufs=2) as pp, \
             tc.tile_pool(name="mp", bufs=4) as mp, \
             tc.tile_pool(name="pf", bufs=1) as pf:
            nan_t = cst.tile([P, 1], f32)
            nc.vector.memset(nan_t[:], float("nan"))
            cdt_t = cst.tile([P, 1], f32)
            nc.sync.dma_start(cdt_t[:], cdt_in[:])
            xls, its = [], []
            _off = 0
            for c, CHCOLS in enumerate(CHUNKS):
                sl = (slice(None), slice(_off, _off + CHCOLS))
                xl_c = pf.tile([P, CHCOLS], f32, tag=f"xl{c}")
                it_c = pf.tile([P, CHCOLS], f32, tag=f"it{c}")
                nc.sync.dma_start(xl_c[:], xl_in[sl])
                nc.sync.dma_start(it_c[:], it_in[sl])
                xls.append(xl_c); its.append(it_c)
                _off += CHCOLS
            cs = 0
            for c, CHCOLS in enumerate(CHUNKS):
                TILES_PER_CHUNK = CHCOLS // PAGES
                xl, it = xls[c], its[c]

                d = pp.tile([P, CHCOLS], f32, tag="d")
                nc.vector.tensor_tensor(d[:], xl[:], it[:], Alu.subtract)
                sg = pp.tile([P, CHCOLS], f32, tag="sg")
                nc.scalar.sign(sg[:], d[:])
                ad = pp.tile([P, CHCOLS], f32, tag="ad")
                nc.scalar.activation(ad[:], d[:], Act.Abs)         # |d|, exact
                A = pp.tile([P, CHCOLS], f32, tag="A")
                nc.vector.tensor_tensor(A[:], xl[:], sg[:], Alu.mult)
                B = pp.tile([P, CHCOLS], f32, tag="B")
                nc.vector.tensor_tensor(B[:], it[:], sg[:], Alu.mult)

                # exact gate as a clamp bound: pol != 0 <=> |d| >= 0.15f.
                # Sign(|d| - C) is sign-exact; {-1,0,1} -> {0,24,48}. The 0.15f
                # midpoint (Sign=0 -> 24) is safe: |d|=C implies n* <= 1.
                g48 = pp.tile([P, CHCOLS], f32, tag="g48")
                nc.scalar.activation(g48[:], ad[:], Act.Copy, bias=-float(CTH))
                nc.scalar.sign(g48[:], g48[:])
                nc.scalar.activation(g48[:], g48[:], Act.Copy, bias=24.0, scale=24.0)
                # window center: round(|d|/C) via the 2^23 trick (+-1 tolerant)
                nr = pp.tile([P, CHCOLS], f32, tag="nr")
                nc.scalar.activation(nr[:], ad[:], Act.Copy, bias=8388608.0,
                                     scale=float(np.float32(1.0) / CTH))
                nc.scalar.activation(nr[:], nr[:], Act.Copy, bias=-8388608.0)

                # three exact boundary checks at k = nr-1, nr, nr+1
                cbs = []
                for ji, jv in enumerate((-1.0, 0.0, 1.0)):
                    kc = pp.tile([P, CHCOLS], f32, tag=f"kc{ji}")
                    nc.vector.tensor_scalar(kc[:], nr[:], jv, float(CTH),
                                            Alu.add, Alu.mult)
                    nc.vector.tensor_tensor(kc[:], kc[:], B[:], Alu.add)
                    nc.vector.tensor_tensor(kc[:], kc[:], A[:], Alu.is_lt)
                    cbs.append(kc)
                ns = pp.tile([P, CHCOLS], f32, tag="ns")
                nc.vector.tensor_tensor(ns[:], cbs[0][:], cbs[1][:], Alu.add)
                npre = pp.tile([P, CHCOLS], f32, tag="npre")
                nc.vector.scalar_tensor_tensor(npre[:], nr[:], -2.0, ns[:],
                                               Alu.add, Alu.add)
                nc.vector.tensor_tensor(npre[:], npre[:], cbs[2][:], Alu.add)
                nstc = pp.tile([P, CHCOLS], f32, tag="nstc")
                nc.vector.tensor_tensor(nstc[:], npre[:], g48[:], Alu.min)

                # S = C*delta_t / |d|  (~2 ulp; garbage when masked-out is fine)
                scr = pp.tile([P, CHCOLS], f32, tag="scr")
                rec = pp.tile([P, CHCOLS], f32, tag="rec")
                nc.vector.reciprocal_approx_accurate(rec[:], ad[:], scr[:])
                S = pp.tile([P, CHCOLS], f32, tag="S")
                nc.scalar.mul(S[:], rec[:], cdt_t[:, 0:1])

                # main pass: one fused DVE op per image row
                for t in range(TILES_PER_CHUNK):
                    o_t = mp.tile([P, PAGES * K], f32, tag="o")
                    S_b = S[:, t * PAGES:(t + 1) * PAGES].broadcast_to([P, PAGES, K])
                    n_b = nstc[:, t * PAGES:(t + 1) * PAGES].broadcast_to([P, PAGES, K])
                    nc.vector._custom_dve(
                        OP, out=o_t[:].rearrange("p (s n) -> p s n", n=K),
                        in0=S_b, in1=n_b, s0=nan_t[:, 0:1], s1=float(-K))
                    o0 = (cs + t * PAGES) * K
                    nc.sync.dma_start(out_d[:, o0:o0 + PAGES * K], o_t[:])
                cs += CHCOLS

    nc.finalize()
    _cache["nc"] = nc
    return nc


def _log_prelude(x, initial_image):
    """xl/It via the same XLA log the reference uses (bit-identical)."""
    import jax, jax.numpy as jnp
    f = _cache.get("logf")
    if f is None:
        f = jax.jit(lambda a: jnp.log(a + EPS))
        _cache["logf"] = f
    It = np.asarray(f(np.asarray(initial_image, dtype=np.float32)))
    xl = np.asarray(f(np.asarray(x, dtype=np.float32)))
    return xl, It


def kernel(x, initial_image, time):
    from concourse.bass_utils import run_bass_kernel_spmd

    nc = _build_bass()
    xl, It = _log_prelude(x, initial_image)
    dt = np.float32(time) - T0
    cdt = np.full((P, 1), CTH * dt, dtype=np.float32)

    xl_f = xl.reshape(-1)
    it_f = It.reshape(-1)
    in_maps = []
    for c in range(NCORES):
        s = c * PIX_PER_CORE
        in_maps.append({
            "xl": xl_f[s:s + PIX_PER_CORE].reshape(P, COLS),
            "itl": it_f[s:s + PIX_PER_CORE].reshape(P, COLS),
            "cdt": cdt,
        })
    res = run_bass_kernel_spmd(nc, in_maps, core_ids=list(range(NCORES)))
    out = np.empty((H * W * CHN, K), dtype=np.float32)
    for c in range(NCORES):
        s = c * PIX_PER_CORE
        out[s:s + PIX_PER_CORE] = res.results[c]["out"].reshape(PIX_PER_CORE, K)
    return out.reshape(H, W, CHN, K)


# revision 21
# speedup vs baseline: 1.0573x; 1.0520x over previous
"""EventCameraSim Trainium2 kernel.

Math: per pixel p with It = log(initial_image+EPS), xl = log(x+EPS),
d = xl - It, the reference emits, for lane k in [1..48]:
    time_events[k] = (pol*k*C)/slope   if the k-th crossing is valid, else NaN
with slope = d/delta_t, pol = sign(d) gated on floor(|d/C|) > 0.

The valid mask reduces to a per-pixel prefix length n*:
  lane k valid  <=>  k <= n*,  where n* counts lanes with
  fl(pol*k*C + It) < xl (pol>0)  /  > xl (pol<0), gated on |d| >= C
(the gate floor(|fl(d/C)|)>0 is exactly equivalent to |d| >= 0.15f in fp32).
n* is computed EXACTLY with 3 boundary checks around ngu = round(|d|/C),
since lanes below/above that window are certain by a margin >> fp noise.
Valid lane values are k * (C*delta_t/|d|) up to a few ulp (tolerance-checked).

Device pipeline per core (90 of 720 rows, flat 345600 px = [128, 2700]):
  1. tiny XLA prelude: xl/It via jnp.log on the same neuron backend as the
     reference (bit-identical logs -> bit-exact NaN mask).
  2. Bass kernel: per-pixel stage (sign/gate/n*/scale on DVE+ACT), then
     45 single-pass custom-DVE ops (2 image rows each) that expand every
     pixel to its 48-lane masked ramp [select(k<=n*, k*S, NaN)] and stream
     66MB/core to HBM. Vector ~190us busy vs ~190us DMA floor per core.

Sharding: H split across 8 cores; no communication.
"""

import sys
import numpy as np

for _p in ("/opt/trn_rl_repo",):
    if _p not in sys.path:
        sys.path.insert(0, _p)

H, W, CHN = 720, 1280, 3
K = 48
CTH = np.float32(0.15)
EPS = np.float32(1e-3)
T0 = np.float32(0.0)
NCORES = 8
ROWS_PER_CORE = H // NCORES          # 90
PIX_PER_CORE = ROWS_PER_CORE * W * CHN  # 345600
P = 128
COLS = PIX_PER_CORE // P             # 2700
CHUNKS = [180, 840, 840, 840]        # cols per chunk (divisible by PAGES; small first chunk hides DMA-in)
NCHUNK = len(CHUNKS)
PAGES = 60                           # pixels per partition-row per output tile (2 image rows)

_cache = {}


def _register_event_op():
    """Register the fused masked-ramp custom DVE op at runtime."""
    from concourse import dve_ops as _dv
    from concourse.dve_spec import (
        Spec, Src0, Src1, C0, C1, One, lower, Idx, PageIdx, select,
    )
    from concourse.dve_uop import DveOpSpec

    for op in _dv.OPS:
        if op.name == "EVENT_RAMP_MASK":
            return op
    k1 = Idx + PageIdx(One, C1)          # C1 = -K at call site -> k = 1..K per page
    body = select(k1 <= Src1, k1 * Src0, C0)

    def ref(in0, in1, s0, s1, imm2):
        x = np.asarray(in0)
        n3 = np.asarray(in1).reshape(x.shape[0], -1, K)
        x3 = x.reshape(x.shape[0], -1, K)
        k = np.arange(1, K + 1, dtype=np.float32)[None, None, :]
        return np.where(k <= n3, k * x3, np.float32(s0)).reshape(x.shape)

    spec = Spec(body=body, reference=ref)
    return _register_op("EVENT_RAMP_MASK", spec, subdim=True)


def _register_op(name, spec, subdim):
    from concourse import dve_ops as _dv
    from concourse.dve_spec import lower, _has_src1
    from concourse.dve_uop import DveOpSpec

    row = _dv._CUSTOM_DVE_ROW_BASE + len(_dv.OPS)
    assert row < 0x20
    _dv._SUB_OPCODE_FOR_NAME[name] = row
    shas = {}
    for ver in ("v3", "v4"):
        s = DveOpSpec(name=name, opcode=row, uops=lower(spec, ver=ver),
                      rd1_en=_has_src1(spec))
        shas[ver] = s.sha(ver)
    op = _dv.DveOp(name, spec, subdim=subdim, uops_sha=shas)
    _dv.OPS.append(op)
    _dv.CUSTOM_DVE_SPECS[name] = spec
    return op


def _register_w3_op():
    """w = ((in0 + s0) * s1) + in1 — the boundary-check lhs, same roundings
    as the reference's karange*CTH + It chain."""
    from concourse import dve_ops as _dv
    from concourse.dve_spec import Spec, Src0, Src1, C0, C1

    for op in _dv.OPS:
        if op.name == "EVENT_W3":
            return op
    spec = Spec(
        body=(Src0 + C0) * C1 + Src1,
        reference=lambda in0, in1, s0, s1, imm2: (
            ((in0 + np.float32(s0)) * np.float32(s1)) + in1).astype(np.float32),
    )
    return _register_op("EVENT_W3", spec, subdim=False)


def _build_bass():
    """Build + finalize the per-core SPMD Bass module (cached)."""
    if "nc" in _cache:
        return _cache["nc"]
    import concourse.bacc as bacc
    import concourse.mybir as mybir
    from concourse.tile import TileContext

    OP = _register_event_op()
    f32 = mybir.dt.float32
    Alu = mybir.AluOpType
    Act = mybir.ActivationFunctionType

    nc = bacc.Bacc()
    xl_in = nc.dram_tensor("xl", [P, COLS], f32, kind="ExternalInput")
    it_in = nc.dram_tensor("itl", [P, COLS], f32, kind="ExternalInput")
    cdt_in = nc.dram_tensor("cdt", [P, 1], f32, kind="ExternalInput")
    out_d = nc.dram_tensor("out", [P, COLS * K], f32, kind="ExternalOutput")

    with TileContext(nc) as tc:
        with tc.tile_pool(name="cst", bufs=1) as cst, \
             tc.tile_pool(name="pp", bufs=2) as pp, \
             tc.tile_pool(name="mp", bufs=4) as mp, \
             tc.tile_pool(name="pf", bufs=1) as pf:
            nan_t = cst.tile([P, 1], f32)
            nc.vector.memset(nan_t[:], float("nan"))
            cdt_t = cst.tile([P, 1], f32)
            nc.sync.dma_start(cdt_t[:], cdt_in[:])
            xls, its = [], []
            _off = 0
            for c, CHCOLS in enumerate(CHUNKS):
                sl = (slice(None), slice(_off, _off + CHCOLS))
                xl_c = pf.tile([P, CHCOLS], f32, tag=f"xl{c}")
                it_c = pf.tile([P, CHCOLS], f32, tag=f"it{c}")
                nc.sync.dma_start(xl_c[:], xl_in[sl])
                nc.sync.dma_start(it_c[:], it_in[sl])
                xls.append(xl_c); its.append(it_c)
                _off += CHCOLS
            cs = 0
            for c, CHCOLS in enumerate(CHUNKS):
                TILES_PER_CHUNK = CHCOLS // PAGES
                xl, it = xls[c], its[c]

                d = pp.tile([P, CHCOLS], f32, tag="d")
                nc.vector.tensor_tensor(d[:], xl[:], it[:], Alu.subtract)
                sg = pp.tile([P, CHCOLS], f32, tag="sg")
                nc.scalar.sign(sg[:], d[:])
                ad = pp.tile([P, CHCOLS], f32, tag="ad")
                nc.scalar.activation(ad[:], d[:], Act.Abs)         # |d|, exact
                A = pp.tile([P, CHCOLS], f32, tag="A")
                nc.vector.tensor_tensor(A[:], xl[:], sg[:], Alu.mult)
                B = pp.tile([P, CHCOLS], f32, tag="B")
                nc.vector.tensor_tensor(B[:], it[:], sg[:], Alu.mult)

                # exact gate as a clamp bound: pol != 0 <=> |d| >= 0.15f.
                # Sign(|d| - C) is sign-exact; {-1,0,1} -> {0,24,48}. The 0.15f
                # midpoint (Sign=0 -> 24) is safe: |d|=C implies n* <= 1.
                g48 = pp.tile([P, CHCOLS], f32, tag="g48")
                nc.scalar.activation(g48[:], ad[:], Act.Copy, bias=-float(CTH))
                nc.scalar.sign(g48[:], g48[:])
                nc.scalar.activation(g48[:], g48[:], Act.Copy, bias=24.0, scale=24.0)
                # window center: round(|d|/C) via the 2^23 trick (+-1 tolerant)
                nr = pp.tile([P, CHCOLS], f32, tag="nr")
                nc.scalar.activation(nr[:], ad[:], Act.Copy, bias=8388608.0,
                                     scale=float(np.float32(1.0) / CTH))
                nc.scalar.activation(nr[:], nr[:], Act.Copy, bias=-8388608.0)

                # three exact boundary checks at k = nr-1, nr, nr+1
                cbs = []
                for ji, jv in enumerate((-1.0, 0.0, 1.0)):
                    kc = pp.tile([P, CHCOLS], f32, tag=f"kc{ji}")
                    nc.vector.tensor_scalar(kc[:], nr[:], jv, float(CTH),
                                            Alu.add, Alu.mult)
                    nc.vector.tensor_tensor(kc[:], kc[:], B[:], Alu.add)
                    nc.vector.tensor_tensor(kc[:], kc[:], A[:], Alu.is_lt)
                    cbs.append(kc)
                ns = pp.tile([P, CHCOLS], f32, tag="ns")
                nc.vector.tensor_tensor(ns[:], cbs[0][:], cbs[1][:], Alu.add)
                npre = pp.tile([P, CHCOLS], f32, tag="npre")
                nc.vector.scalar_tensor_tensor(npre[:], nr[:], -2.0, ns[:],
                                               Alu.add, Alu.add)
                nc.vector.tensor_tensor(npre[:], npre[:], cbs[2][:], Alu.add)
                nstc = pp.tile([P, CHCOLS], f32, tag="nstc")
                nc.vector.tensor_tensor(nstc[:], npre[:], g48[:], Alu.min)

                # S = C*delta_t / |d|  (~2 ulp; garbage when masked-out is fine)
                scr = pp.tile([P, CHCOLS], f32, tag="scr")
                rec = pp.tile([P, CHCOLS], f32, tag="rec")
                nc.vector.reciprocal_approx_accurate(rec[:], ad[:], scr[:])
                S = pp.tile([P, CHCOLS], f32, tag="S")
                nc.scalar.mul(S[:], rec[:], cdt_t[:, 0:1])

                # main pass: one fused DVE op per image row
                for t in range(TILES_PER_CHUNK):
                    o_t = mp.tile([P, PAGES * K], f32, tag="o")
                    S_b = S[:, t * PAGES:(t + 1) * PAGES].broadcast_to([P, PAGES, K])
                    n_b = nstc[:, t * PAGES:(t + 1) * PAGES].broadcast_to([P, PAGES, K])
                    nc.vector._custom_dve(
                        OP, out=o_t[:].rearrange("p (s n) -> p s n", n=K),
                        in0=S_b, in1=n_b, s0=nan_t[:, 0:1], s1=float(-K))
                    o0 = (cs + t * PAGES) * K
                    nc.sync.dma_start(out_d[:, o0:o0 + PAGES * K], o_t[:])
                cs += CHCOLS

    nc.finalize()
    _cache["nc"] = nc
    return nc


def _log_prelude(x, initial_image):
    """xl/It via the same XLA log the reference uses (bit-identical)."""
    import jax, jax.numpy as jnp
    f = _cache.get("logf")
    if f is None:
        f = jax.jit(lambda a: jnp.log(a + EPS))
        _cache["logf"] = f
    It = np.asarray(f(np.asarray(initial_image, dtype=np.float32)))
    xl = np.asarray(f(np.asarray(x, dtype=np.float32)))
    return xl, It


def kernel(x, initial_image, time):
    from concourse.bass_utils import run_bass_kernel_spmd

    nc = _build_bass()
    xl, It = _log_prelude(x, initial_image)
    dt = np.float32(time) - T0
    cdt = np.full((P, 1), CTH * dt, dtype=np.float32)

    xl_f = xl.reshape(-1)
    it_f = It.reshape(-1)
    in_maps = []
    for c in range(NCORES):
        s = c * PIX_PER_CORE
        in_maps.append({
            "xl": xl_f[s:s + PIX_PER_CORE].reshape(P, COLS),
            "itl": it_f[s:s + PIX_PER_CORE].reshape(P, COLS),
            "cdt": cdt,
        })
    res = run_bass_kernel_spmd(nc, in_maps, core_ids=list(range(NCORES)))
    out = np.empty((H * W * CHN, K), dtype=np.float32)
    for c in range(NCORES):
        s = c * PIX_PER_CORE
        out[s:s + PIX_PER_CORE] = res.results[c]["out"].reshape(PIX_PER_CORE, K)
    return out.reshape(H, W, CHN, K)


# revision 22
# speedup vs baseline: 1.0615x; 1.0040x over previous
"""EventCameraSim Trainium2 kernel.

Math: per pixel p with It = log(initial_image+EPS), xl = log(x+EPS),
d = xl - It, the reference emits, for lane k in [1..48]:
    time_events[k] = (pol*k*C)/slope   if the k-th crossing is valid, else NaN
with slope = d/delta_t, pol = sign(d) gated on floor(|d/C|) > 0.

The valid mask reduces to a per-pixel prefix length n*:
  lane k valid  <=>  k <= n*,  where n* counts lanes with
  fl(pol*k*C + It) < xl (pol>0)  /  > xl (pol<0), gated on |d| >= C
(the gate floor(|fl(d/C)|)>0 is exactly equivalent to |d| >= 0.15f in fp32).
n* is computed EXACTLY with 3 boundary checks around ngu = round(|d|/C),
since lanes below/above that window are certain by a margin >> fp noise.
Valid lane values are k * (C*delta_t/|d|) up to a few ulp (tolerance-checked).

Device pipeline per core (90 of 720 rows, flat 345600 px = [128, 2700]):
  1. tiny XLA prelude: xl/It via jnp.log on the same neuron backend as the
     reference (bit-identical logs -> bit-exact NaN mask).
  2. Bass kernel: per-pixel stage (sign/gate/n*/scale on DVE+ACT), then
     45 single-pass custom-DVE ops (2 image rows each) that expand every
     pixel to its 48-lane masked ramp [select(k<=n*, k*S, NaN)] and stream
     66MB/core to HBM. Vector ~190us busy vs ~190us DMA floor per core.

Sharding: H split across 8 cores; no communication.
"""

import sys
import numpy as np

for _p in ("/opt/trn_rl_repo",):
    if _p not in sys.path:
        sys.path.insert(0, _p)

H, W, CHN = 720, 1280, 3
K = 48
CTH = np.float32(0.15)
EPS = np.float32(1e-3)
T0 = np.float32(0.0)
NCORES = 8
ROWS_PER_CORE = H // NCORES          # 90
PIX_PER_CORE = ROWS_PER_CORE * W * CHN  # 345600
P = 128
COLS = PIX_PER_CORE // P             # 2700
CHUNKS = [180, 840, 840, 840]        # cols per chunk (divisible by PAGES; small first chunk hides DMA-in)
NCHUNK = len(CHUNKS)
PAGES = 60                           # pixels per partition-row per output tile (2 image rows)

_cache = {}


def _register_event_op():
    """Register the fused masked-ramp custom DVE op at runtime."""
    from concourse import dve_ops as _dv
    from concourse.dve_spec import (
        Spec, Src0, Src1, C0, C1, One, lower, Idx, PageIdx, select,
    )
    from concourse.dve_uop import DveOpSpec

    for op in _dv.OPS:
        if op.name == "EVENT_RAMP_MASK":
            return op
    k1 = Idx + PageIdx(One, C1)          # C1 = -K at call site -> k = 1..K per page
    body = select(k1 <= Src1, k1 * Src0, C0)

    def ref(in0, in1, s0, s1, imm2):
        x = np.asarray(in0)
        n3 = np.asarray(in1).reshape(x.shape[0], -1, K)
        x3 = x.reshape(x.shape[0], -1, K)
        k = np.arange(1, K + 1, dtype=np.float32)[None, None, :]
        return np.where(k <= n3, k * x3, np.float32(s0)).reshape(x.shape)

    spec = Spec(body=body, reference=ref)
    return _register_op("EVENT_RAMP_MASK", spec, subdim=True)


def _register_op(name, spec, subdim):
    from concourse import dve_ops as _dv
    from concourse.dve_spec import lower, _has_src1
    from concourse.dve_uop import DveOpSpec

    row = _dv._CUSTOM_DVE_ROW_BASE + len(_dv.OPS)
    assert row < 0x20
    _dv._SUB_OPCODE_FOR_NAME[name] = row
    shas = {}
    for ver in ("v3", "v4"):
        s = DveOpSpec(name=name, opcode=row, uops=lower(spec, ver=ver),
                      rd1_en=_has_src1(spec))
        shas[ver] = s.sha(ver)
    op = _dv.DveOp(name, spec, subdim=subdim, uops_sha=shas)
    _dv.OPS.append(op)
    _dv.CUSTOM_DVE_SPECS[name] = spec
    return op


def _register_w3_op():
    """w = ((in0 + s0) * s1) + in1 — the boundary-check lhs, same roundings
    as the reference's karange*CTH + It chain."""
    from concourse import dve_ops as _dv
    from concourse.dve_spec import Spec, Src0, Src1, C0, C1

    for op in _dv.OPS:
        if op.name == "EVENT_W3":
            return op
    spec = Spec(
        body=(Src0 + C0) * C1 + Src1,
        reference=lambda in0, in1, s0, s1, imm2: (
            ((in0 + np.float32(s0)) * np.float32(s1)) + in1).astype(np.float32),
    )
    return _register_op("EVENT_W3", spec, subdim=False)


def _build_bass():
    """Build + finalize the per-core SPMD Bass module (cached)."""
    if "nc" in _cache:
        return _cache["nc"]
    import concourse.bacc as bacc
    import concourse.mybir as mybir
    from concourse.tile import TileContext

    OP = _register_event_op()
    f32 = mybir.dt.float32
    Alu = mybir.AluOpType
    Act = mybir.ActivationFunctionType

    nc = bacc.Bacc()
    xl_in = nc.dram_tensor("xl", [P, COLS], f32, kind="ExternalInput")
    it_in = nc.dram_tensor("itl", [P, COLS], f32, kind="ExternalInput")
    cdt_in = nc.dram_tensor("cdt", [P, 1], f32, kind="ExternalInput")
    out_d = nc.dram_tensor("out", [P, COLS * K], f32, kind="ExternalOutput")

    with TileContext(nc) as tc:
        with tc.tile_pool(name="cst", bufs=1) as cst, \
             tc.tile_pool(name="pp", bufs=2) as pp, \
             tc.tile_pool(name="mp", bufs=4) as mp, \
             tc.tile_pool(name="pf", bufs=1) as pf:
            nan_t = cst.tile([P, 1], f32)
            nc.vector.memset(nan_t[:], float("nan"))
            cdt_t = cst.tile([P, 1], f32)
            nc.sync.dma_start(cdt_t[:], cdt_in[:])
            xls, its = [], []
            _off = 0
            for c, CHCOLS in enumerate(CHUNKS):
                sl = (slice(None), slice(_off, _off + CHCOLS))
                xl_c = pf.tile([P, CHCOLS], f32, tag=f"xl{c}")
                it_c = pf.tile([P, CHCOLS], f32, tag=f"it{c}")
                nc.sync.dma_start(xl_c[:], xl_in[sl])
                nc.sync.dma_start(it_c[:], it_in[sl])
                xls.append(xl_c); its.append(it_c)
                _off += CHCOLS
            cs = 0

            def pp_gen(c, state):
                """Per-pixel stage for chunk c, one instruction per yield so
                it can be interleaved between main-pass ops of chunk c-1."""
                CHCOLS = CHUNKS[c]
                xl, it = xls[c], its[c]
                d = pp.tile([P, CHCOLS], f32, tag="d")
                nc.vector.tensor_tensor(d[:], xl[:], it[:], Alu.subtract); yield
                sg = pp.tile([P, CHCOLS], f32, tag="sg")
                nc.scalar.sign(sg[:], d[:]); yield
                ad = pp.tile([P, CHCOLS], f32, tag="ad")
                nc.scalar.activation(ad[:], d[:], Act.Abs); yield   # |d|, exact
                A = pp.tile([P, CHCOLS], f32, tag="A")
                nc.vector.tensor_tensor(A[:], xl[:], sg[:], Alu.mult); yield
                B = pp.tile([P, CHCOLS], f32, tag="B")
                nc.vector.tensor_tensor(B[:], it[:], sg[:], Alu.mult); yield
                # window center: round(|d|/C) via the 2^23 trick (+-1 tolerant)
                nr = pp.tile([P, CHCOLS], f32, tag="nr")
                nc.scalar.activation(nr[:], ad[:], Act.Copy, bias=8388608.0,
                                     scale=float(np.float32(1.0) / CTH)); yield
                nc.scalar.activation(nr[:], nr[:], Act.Copy, bias=-8388608.0); yield
                # exact gate as a clamp bound: pol != 0 <=> |d| >= 0.15f.
                # Sign(|d| - C) is sign-exact; {-1,0,1} -> {0,24,48}. The 0.15f
                # midpoint (Sign=0 -> 24) is safe: |d|=C implies n* <= 1.
                g48 = pp.tile([P, CHCOLS], f32, tag="g48")
                nc.scalar.activation(g48[:], ad[:], Act.Copy, bias=-float(CTH)); yield
                nc.scalar.sign(g48[:], g48[:]); yield
                nc.scalar.activation(g48[:], g48[:], Act.Copy, bias=24.0, scale=24.0); yield
                # three exact boundary checks at k = nr-1, nr, nr+1
                cbs = []
                for ji, jv in enumerate((-1.0, 0.0, 1.0)):
                    kc = pp.tile([P, CHCOLS], f32, tag=f"kc{ji}")
                    nc.vector.tensor_scalar(kc[:], nr[:], jv, float(CTH),
                                            Alu.add, Alu.mult); yield
                    nc.vector.tensor_tensor(kc[:], kc[:], B[:], Alu.add); yield
                    nc.vector.tensor_tensor(kc[:], kc[:], A[:], Alu.is_lt); yield
                    cbs.append(kc)
                ns = pp.tile([P, CHCOLS], f32, tag="ns")
                nc.vector.tensor_tensor(ns[:], cbs[0][:], cbs[1][:], Alu.add); yield
                npre = pp.tile([P, CHCOLS], f32, tag="npre")
                nc.vector.scalar_tensor_tensor(npre[:], nr[:], -2.0, ns[:],
                                               Alu.add, Alu.add); yield
                nc.vector.tensor_tensor(npre[:], npre[:], cbs[2][:], Alu.add); yield
                nstc = pp.tile([P, CHCOLS], f32, tag="nstc")
                nc.vector.tensor_tensor(nstc[:], npre[:], g48[:], Alu.min); yield
                # S = C*delta_t / |d|  (~2 ulp; garbage when masked-out is fine)
                scr = pp.tile([P, CHCOLS], f32, tag="scr")
                rec = pp.tile([P, CHCOLS], f32, tag="rec")
                nc.vector.reciprocal_approx_fast(scr[:], ad[:]); yield
                from concourse.dve_ops import RECIPROCAL_APPROX_NR
                nc.vector._custom_dve(RECIPROCAL_APPROX_NR, out=rec[:], in0=ad[:],
                                      in1=scr[:], s0=2.0); yield
                S = pp.tile([P, CHCOLS], f32, tag="S")
                nc.scalar.mul(S[:], rec[:], cdt_t[:, 0:1])
                state[c] = (S, nstc)
                yield

            state = {}
            for _ in pp_gen(0, state):
                pass
            for c, CHCOLS in enumerate(CHUNKS):
                TILES_PER_CHUNK = CHCOLS // PAGES
                S, nstc = state[c]
                nxt = pp_gen(c + 1, state) if c + 1 < NCHUNK else iter(())
                # main pass: one fused DVE op per 2 image rows, interleaved
                # with the next chunk's per-pixel stage so output-tile
                # production (and thus the out-DMA stream) never pauses.
                for t in range(TILES_PER_CHUNK):
                    o_t = mp.tile([P, PAGES * K], f32, tag="o")
                    S_b = S[:, t * PAGES:(t + 1) * PAGES].broadcast_to([P, PAGES, K])
                    n_b = nstc[:, t * PAGES:(t + 1) * PAGES].broadcast_to([P, PAGES, K])
                    nc.vector._custom_dve(
                        OP, out=o_t[:].rearrange("p (s n) -> p s n", n=K),
                        in0=S_b, in1=n_b, s0=nan_t[:, 0:1], s1=float(-K))
                    o0 = (cs + t * PAGES) * K
                    nc.sync.dma_start(out_d[:, o0:o0 + PAGES * K], o_t[:])
                    next(nxt, None)
                    next(nxt, None)
                for _ in nxt:
                    pass
                cs += CHCOLS

    nc.finalize()
    _cache["nc"] = nc
    return nc


def _log_prelude(x, initial_image):
    """xl/It via the same XLA log the reference uses (bit-identical)."""
    import jax, jax.numpy as jnp
    f = _cache.get("logf")
    if f is None:
        f = jax.jit(lambda a: jnp.log(a + EPS))
        _cache["logf"] = f
    It = np.asarray(f(np.asarray(initial_image, dtype=np.float32)))
    xl = np.asarray(f(np.asarray(x, dtype=np.float32)))
    return xl, It


def kernel(x, initial_image, time):
    from concourse.bass_utils import run_bass_kernel_spmd

    nc = _build_bass()
    xl, It = _log_prelude(x, initial_image)
    dt = np.float32(time) - T0
    cdt = np.full((P, 1), CTH * dt, dtype=np.float32)

    xl_f = xl.reshape(-1)
    it_f = It.reshape(-1)
    in_maps = []
    for c in range(NCORES):
        s = c * PIX_PER_CORE
        in_maps.append({
            "xl": xl_f[s:s + PIX_PER_CORE].reshape(P, COLS),
            "itl": it_f[s:s + PIX_PER_CORE].reshape(P, COLS),
            "cdt": cdt,
        })
    res = run_bass_kernel_spmd(nc, in_maps, core_ids=list(range(NCORES)))
    out = np.empty((H * W * CHN, K), dtype=np.float32)
    for c in range(NCORES):
        s = c * PIX_PER_CORE
        out[s:s + PIX_PER_CORE] = res.results[c]["out"].reshape(PIX_PER_CORE, K)
    return out.reshape(H, W, CHN, K)
